# revision 60
# baseline (speedup 1.0000x reference)
"""nn_BoundaryGuidedDSTLayer Trainium2 Bass kernel (8-core SPMD, no collectives).

Sharding: core c = (b = c//2, half = c%2). Each core computes the conv
pre-mix + LN1 + K/V over the full T of its batch (needed for full
attention), and Q / attention / out-proj / MLP / DSA only for its local
1024-column half.

v2 layout/engine strategy:
- All matmul operands bf16 (fp32 PSUM accumulate); weights cast on host.
- LN stats stacked on psum partitions via one-hot (1/C) stationary columns
  so the row math runs lane-parallel; 1/sigma = exp(-0.5*ln(var+eps)) and
  softmax 1/denom via DVE reciprocal_approx_fast, keeping every ACT
  transcendental in just two table sets (gelu | natural_log_exp).
- Attention: per (head, key-tile) pipeline scores->exp->AV with 2-deep
  psum buffering; denominator handled off the critical path so psum
  recycles without stalling the PE.
- DSA branch scheduled in the K/V/Q window (its gelu+pointwise ride the
  MLP phase, accumulated straight into the MLP2 psum banks).
"""
import sys, os

for _p in ("/opt/trn_rl_repo",):
    if os.path.isdir(_p) and _p not in sys.path:
        sys.path.append(_p)

import numpy as np
import ml_dtypes
import concourse.bass as bass
import concourse.mybir as mybir
import concourse.tile as tile
from concourse.bacc import Bacc
from concourse.bass_utils import run_bass_kernel_spmd

dt = mybir.dt
F32, F32R, U32, BF16 = dt.float32, dt.float32r, dt.uint32, dt.bfloat16
AF = mybir.ActivationFunctionType
OP = mybir.AluOpType
BF = ml_dtypes.bfloat16
F8 = ml_dtypes.float8_e4m3

P = 128
B, T, C, H = 4, 2048, 512, 8
HD = C // H          # 64
FF = 4 * C           # 2048
TL = T // 2          # 1024 local columns per core
CK = C // P          # 4
FFK = FF // P        # 16
NCH = T // 512       # 4 chunks over full T
NL = TL // 512       # 2 chunks over local T
TK = T // P          # 16 key tiles

_CACHED = None
DEBUG = False


def _build():
    nc = Bacc("TRN2", target_bir_lowering=False, debug=False, num_devices=8)

    # ---- DRAM I/O ----
    d_xT = nc.dram_tensor("xT", [C, T + 2], BF16, kind="ExternalInput")
    d_xd = nc.dram_tensor("xd", [C, TL + 2], BF16, kind="ExternalInput")
    d_A = nc.dram_tensor("Arow", [1, T], BF16, kind="ExternalInput")
    d_qA = nc.dram_tensor("qArow", [1, TL], BF16, kind="ExternalInput")
    d_mask = nc.dram_tensor("maskbc", [P, 2], BF16, kind="ExternalInput")
    d_qoff = nc.dram_tensor("qoff", [1, 1], U32, kind="ExternalInput")
    d_convw = nc.dram_tensor("convw", [3, C, C], BF16, kind="ExternalInput")
    d_convb = nc.dram_tensor("convb", [C], F32, kind="ExternalInput")
    d_wqkv = nc.dram_tensor("wqkv", [C, 3 * C], BF16, kind="ExternalInput")
    d_wo = nc.dram_tensor("wo", [HD, H, C], BF16, kind="ExternalInput")
    d_ob = nc.dram_tensor("ob", [C], F32, kind="ExternalInput")
    d_w1 = nc.dram_tensor("w1", [C, FF], BF16, kind="ExternalInput")
    d_b1 = nc.dram_tensor("b1", [FF], F32, kind="ExternalInput")
    d_w2 = nc.dram_tensor("w2", [FF, C], BF16, kind="ExternalInput")
    d_bfin = nc.dram_tensor("bfin", [C], F32, kind="ExternalInput")
    d_pw = nc.dram_tensor("pw", [C, C], BF16, kind="ExternalInput")
    d_dsag = nc.dram_tensor("dsag", [C], F32, kind="ExternalInput")
    d_dsab = nc.dram_tensor("dsab", [C], F32, kind="ExternalInput")
    d_dw3 = nc.dram_tensor("dw3", [C, 3], F32, kind="ExternalInput")
    d_dsadb = nc.dram_tensor("dsadb", [C], F32, kind="ExternalInput")
    d_statOH = nc.dram_tensor("statOH", [P, 4, P], BF16, kind="ExternalInput")
    d_ones8 = nc.dram_tensor("ones8", [P, P], F32, kind="ExternalInput")
    d_eps8 = nc.dram_tensor("eps8", [P, 1], F32, kind="ExternalInput")
    d_out = nc.dram_tensor("outT", [C, TL], F32, kind="ExternalOutput")
    if DEBUG:
        d_dbg_hat = nc.dram_tensor("dbg_hat", [C, T], BF16, kind="ExternalOutput")
        d_dbg_k = nc.dram_tensor("dbg_k", [HD + 1, H, T], BF16, kind="ExternalOutput")
        d_dbg_q = nc.dram_tensor("dbg_q", [HD + 1, H, TL], BF16, kind="ExternalOutput")
        d_dbg_v = nc.dram_tensor("dbg_v", [P, TK, H, HD + 1], BF16, kind="ExternalOutput")
        d_dbg_at = nc.dram_tensor("dbg_at", [HD, H, TL], BF16, kind="ExternalOutput")
        d_dbg_f2 = nc.dram_tensor("dbg_f2", [C, TL], BF16, kind="ExternalOutput")
        d_dbg_z1 = nc.dram_tensor("dbg_z1", [C, TL], BF16, kind="ExternalOutput")

    eng = nc.vector

    with tile.TileContext(nc) as tc, nc.allow_low_precision(
            reason="bf16 matmuls/activations; rel-err budget 2e-2"):
        # ---------- persistent small pools ----------
        # right side: pools that live to the end (LIFO release at exit);
        # left side: phase-scoped pools released as phases retire.
        consts = tc.alloc_tile_pool(name="consts", bufs=1, side="right")
        qoff_s = consts.tile([1, 1], U32, tag="qoff")
        nc.sync.dma_start(out=qoff_s, in_=d_qoff[:, :])
        regs = nc.alloc_registers("qoffr")
        nc.regs_load(regs, qoff_s[0:1, 0:1])
        j0 = nc.snap(regs, donate=True, min_val=0, max_val=TL)
        statOH = consts.tile([P, 4, P], BF16, tag="statOH")
        ones8_r = consts.tile([P, P], F32R, tag="ones8")

        def load_consts():
            nc.sync.dma_start(out=statOH, in_=d_statOH[:, :, :])
            nc.sync.dma_start(out=ones8_r, in_=d_ones8[:, :].bitcast(F32R))
        eps8 = consts.tile([P, 1], F32, tag="eps8")
        convb_s = consts.tile([P, CK], F32, tag="convb")
        ob_s = consts.tile([P, CK], F32, tag="ob")
        b1_s = consts.tile([P, FFK], F32, tag="b1")
        bfin_s = consts.tile([P, CK], F32, tag="bfin")
        dsag_s = consts.tile([P, CK], F32, tag="dsag")
        dsab_s = consts.tile([P, CK], F32, tag="dsab")
        dw3_s = consts.tile([P, CK, 3], F32, tag="dw3")
        dsadb_s = consts.tile([P, CK], F32, tag="dsadb")
        mask_s = consts.tile([P, 2], BF16, tag="mask")

        def load_consts2():
            nc.sync.dma_start(out=eps8, in_=d_eps8[:, :])
            nc.sync.dma_start(out=convb_s,
                              in_=d_convb.rearrange("(m p) -> p m", p=P))
            nc.sync.dma_start(out=ob_s, in_=d_ob.rearrange("(m p) -> p m", p=P))
            nc.sync.dma_start(out=b1_s, in_=d_b1.rearrange("(m p) -> p m", p=P))
            nc.sync.dma_start(out=bfin_s,
                              in_=d_bfin.rearrange("(m p) -> p m", p=P))
            nc.sync.dma_start(out=dsag_s,
                              in_=d_dsag.rearrange("(m p) -> p m", p=P))
            nc.sync.dma_start(out=dsab_s,
                              in_=d_dsab.rearrange("(m p) -> p m", p=P))
            nc.sync.dma_start(out=dw3_s,
                              in_=d_dw3.rearrange("(m p) d -> p m d", p=P))
            nc.sync.dma_start(out=dsadb_s,
                              in_=d_dsadb.rearrange("(m p) -> p m", p=P))
            nc.sync.dma_start(out=mask_s, in_=d_mask[:, :])

        # ---------- persistent activation state ----------
        row_pool = tc.alloc_tile_pool(name="rows", bufs=1, side="right")
        tmp_pool = tc.alloc_tile_pool(name="tmp", bufs=2, side="right")
        bc_pool = tc.alloc_tile_pool(name="bcs", bufs=2, side="right")
        sq_pool = tc.alloc_tile_pool(name="sqp", bufs=4, side="right")
        dsa_pool = tc.alloc_tile_pool(name="dsap", bufs=1, side="right")
        xw = dsa_pool.tile([P, CK, TL + 2], BF16, tag="xw")
        z1_pool = tc.alloc_tile_pool(name="z1p", bufs=1, side="right")
        z1_s = z1_pool.tile([P, CK, TL], BF16, tag="z1")

        # left stack (phase-scoped): kvst at the bottom (lives to end of
        # attention), then the conv/KVQ pools that retire before it.
        kv_state = tc.alloc_tile_pool(name="kvst", bufs=1, side="left")
        kaug = kv_state.tile([HD + 1, H, T], BF16, tag="kaug")
        qaug = kv_state.tile([HD + 1, H, TL], BF16, tag="qaug")
        vsb = kv_state.tile([P, TK, H, HD + 1], BF16, tag="v")

        hat_pool = tc.alloc_tile_pool(name="hatp", bufs=1, side="left")
        hat = hat_pool.tile([P, CK, T], BF16, tag="hat")

        # =================== Phase 1: conv + gelu + residual + stats =========
        p1 = tc.alloc_tile_pool(name="p1", bufs=1, side="left")
        x_sb = p1.tile([P, CK, T + 2], BF16, tag="xsb")
        convw_s = p1.tile([P, 3, CK, C], BF16, tag="convw")
        nc.sync.dma_start(
            out=x_sb[:, :, 0:514],
            in_=d_xT.rearrange("(k p) t -> p k t", p=P)[:, :, 0:514])
        nc.sync.dma_start(out=convw_s,
                          in_=d_convw.rearrange("d (k p) o -> p d k o", p=P))
        for n in range(1, NCH):
            lo = 512 * n
            hi_ = T + 2 if n == NCH - 1 else lo + 514
            nc.sync.dma_start(
                out=x_sb[:, :, lo:hi_],
                in_=d_xT.rearrange("(k p) t -> p k t", p=P)[:, :, lo:hi_])
        load_consts()
        load_consts2()
        nc.sync.dma_start(out=xw, in_=d_xd.rearrange("(k p) t -> p k t", p=P))
        nc.gpsimd.memset(vsb[:, :, :, HD:HD + 1], 1.0)
        for h in range(H):
            nc.sync.dma_start(out=kaug[HD:HD + 1, h, :], in_=d_A[:, :])
            nc.sync.dma_start(out=qaug[HD:HD + 1, h, :], in_=d_qA[:, :])
        g_pool = tc.alloc_tile_pool(name="gp", bufs=3, side="left")
        psA = tc.alloc_tile_pool(name="psA", bufs=2, space="PSUM")
        psM = tc.alloc_tile_pool(name="psM", bufs=1, space="PSUM")
        psE = tc.alloc_tile_pool(name="psE", bufs=1, space="PSUM")

        # LN stats land on psum partition 32*chunk via one-hot stationary
        # columns (matmul operand base partitions are limited to 0/32/64,
        # so chunk 3 goes to a second tile at partition 0).
        ps_m = psM.tile([P, 512], F32, tag="m1")
        ps_e = psE.tile([P, 512], F32, tag="e1")
        ps_mB = psM.tile([P, 512], F32, tag="m1b")
        ps_eB = psE.tile([P, 512], F32, tag="e1b")

        def emit_stats(n, mo, sq_t):
            """stats matmuls for sub-block (n, mo); deferred so their inputs
            are ready when the in-order PE queue reaches them."""
            c0 = 512 * n
            ps_mx, ps_ex = (ps_m, ps_e) if n < 3 else (ps_mB, ps_eB)
            oh = statOH[:, n if n < 3 else 0, :]
            nc.tensor.matmul(ps_mx, oh, hat[:, mo, c0:c0 + 512],
                             start=(mo == 0 and n in (0, 3)),
                             stop=(mo == CK - 1 and n in (2, 3)))
            nc.tensor.matmul(ps_ex, oh, sq_t,
                             start=(mo == 0 and n in (0, 3)),
                             stop=(mo == CK - 1 and n in (2, 3)))

        pend = []
        for n in range(NCH):
            c0 = 512 * n
            for mo in range(CK):
                ps_c = psA.tile([P, 512], F32, tag="mm")
                first = True
                for dtap in range(3):
                    for kc in range(CK):
                        nc.tensor.matmul(
                            ps_c,
                            convw_s[:, dtap, kc, mo * P:(mo + 1) * P],
                            x_sb[:, kc, c0 + dtap:c0 + dtap + 512],
                            start=first, stop=(dtap == 2 and kc == CK - 1),
                        )
                        first = False
                if len(pend) >= 2:
                    emit_stats(*pend.pop(0))
                g_t = g_pool.tile([P, 512], BF16, tag="g")
                nc.scalar.activation(out=g_t, in_=ps_c, func=AF.Gelu,
                                     bias=convb_s[:, mo:mo + 1])
                eng.tensor_tensor(out=hat[:, mo, c0:c0 + 512], in0=g_t,
                                  in1=x_sb[:, mo, c0 + 1:c0 + 513], op=OP.add)
                sq_t = sq_pool.tile([P, 512], BF16, tag="sq")
                nc.scalar.activation(out=sq_t, in_=hat[:, mo, c0:c0 + 512],
                                     func=AF.Square)
                pend.append((n, mo, sq_t))
        for args in pend:
            emit_stats(*args)

        def ln_rows(ps_m, ps_e, tag=""):
            """psum [P,512] mean + ex2 (rows at 32*chunk) -> (r, mr) f32r rows.
            1/sigma = sqrt(1/(var+eps)): DVE fast reciprocal + ACT Sqrt keeps
            the ACT table in the sqrt set (no ln/exp set thrash)."""
            m_s = row_pool.tile([P, 512], F32, tag=f"m{tag}")
            eng.tensor_copy(out=m_s, in_=ps_m)
            v_s = row_pool.tile([P, 512], F32, tag=f"v{tag}")
            eng.tensor_tensor(out=v_s, in0=m_s, in1=m_s, op=OP.mult)
            eng.scalar_tensor_tensor(out=v_s, in0=ps_e, scalar=eps8[:, 0:1],
                                     in1=v_s, op0=OP.add, op1=OP.subtract)
            vr_s = row_pool.tile([P, 512], F32, tag=f"vr{tag}")
            eng.reciprocal_approx_fast(out=vr_s, in_=v_s)
            r_s = row_pool.tile([P, 512], F32R, tag=f"r{tag}")
            nc.scalar.activation(out=r_s, in_=vr_s, func=AF.Sqrt)
            mr_s = row_pool.tile([P, 512], F32R, tag=f"mr{tag}")
            eng.tensor_tensor(out=mr_s, in0=m_s, in1=r_s.bitcast(F32),
                              op=OP.mult)
            return r_s, mr_s

        def bcast_rows(psB, r_s, mr_s, n, w):
            """broadcast chunk-n rows (partition 32n) of (r, mr) to bf16 [P,1024]."""
            p0 = 32 * n
            ps_bc = psB.tile([P, 1024], F32, tag="bc")
            nc.tensor.matmul(ps_bc[:, 0:w], ones8_r[p0:p0 + 1, :],
                             r_s[p0:p0 + 1, 0:w],
                             start=True, stop=True)
            nc.tensor.matmul(ps_bc[:, 512:512 + w], ones8_r[p0:p0 + 1, :],
                             mr_s[p0:p0 + 1, 0:w],
                             start=True, stop=True)
            bc = bc_pool.tile([P, 1024], BF16, tag="bc")
            eng.tensor_copy(out=bc[:, 0:w], in_=ps_bc[:, 0:w])
            eng.tensor_copy(out=bc[:, 512:512 + w], in_=ps_bc[:, 512:512 + w])
            return bc

        r1_s, mr1_s = ln_rows(ps_m, ps_e)
        r1b_s, mr1b_s = ln_rows(ps_mB, ps_eB, tag="b")

        # ============ Phase 2: normalize hat + K/V per chunk, then Q =========
        psB = tc.alloc_tile_pool(name="psB", bufs=1, space="PSUM")
        p2 = tc.alloc_tile_pool(name="p2", bufs=1, side="left")
        wqkv_s = p2.tile([P, CK, 3 * C], BF16, tag="wqkv")
        nc.sync.dma_start(out=wqkv_s,
                          in_=d_wqkv.rearrange("(k p) o -> p k o", p=P))
        st_pool = tc.alloc_tile_pool(name="stage", bufs=3, side="left")

        for n in range(NCH):
            c0 = 512 * n
            if n < 3:
                bc = bcast_rows(psB, r1_s, mr1_s, n, 512)
            else:
                bc = bcast_rows(psB, r1b_s, mr1b_s, 0, 512)
            for kc in range(CK):
                t_s = tmp_pool.tile([P, 512], BF16, tag="t")
                eng.tensor_tensor(out=t_s, in0=hat[:, kc, c0:c0 + 512],
                                  in1=bc[:, 0:512], op=OP.mult)
                eng.tensor_tensor(out=hat[:, kc, c0:c0 + 512], in0=t_s,
                                  in1=bc[:, 512:1024], op=OP.subtract)
            # K tiles for this chunk
            for mo in range(CK):
                ps_k = psA.tile([P, 512], F32, tag="mm")
                for kc in range(CK):
                    nc.tensor.matmul(ps_k, wqkv_s[:, kc, C + mo * P:C + (mo + 1) * P],
                                     hat[:, kc, c0:c0 + 512],
                                     start=(kc == 0), stop=(kc == CK - 1))
                st = st_pool.tile([P, 512], BF16, tag="kst")
                nc.scalar.activation(out=st, in_=ps_k, func=AF.Copy)
                nc.sync.dma_start(out=kaug[0:HD, 2 * mo, c0:c0 + 512], in_=st[0:HD, :])
                nc.sync.dma_start(out=kaug[0:HD, 2 * mo + 1, c0:c0 + 512], in_=st[HD:P, :])
            # V tiles (natural token-major layout)
            for tt in range(4):
                g = 4 * n + tt
                ps_v = psA.tile([P, 512], F32, tag="mm")
                for kc in range(CK):
                    nc.tensor.matmul(ps_v, hat[:, kc, c0 + tt * P:c0 + (tt + 1) * P],
                                     wqkv_s[:, kc, 2 * C:3 * C],
                                     start=(kc == 0), stop=(kc == CK - 1))
                nc.scalar.activation(
                    out=vsb[:, g, :, 0:HD],
                    in_=ps_v.rearrange("p (h d) -> p h d", d=HD), func=AF.Copy)
        # Q tiles (local half via dynamic offset)
        for mo in range(CK):
            for n2 in range(NL):
                ps_q = psA.tile([P, 512], F32, tag="mm")
                for kc in range(CK):
                    nc.tensor.matmul(ps_q, wqkv_s[:, kc, mo * P:(mo + 1) * P],
                                     hat[:, kc, bass.ds(j0 + n2 * 512, 512)],
                                     start=(kc == 0), stop=(kc == CK - 1))
                st = st_pool.tile([P, 512], BF16, tag="kst")
                nc.scalar.activation(out=st, in_=ps_q, func=AF.Copy)
                nc.sync.dma_start(out=qaug[0:HD, 2 * mo, n2 * 512:(n2 + 1) * 512],
                                  in_=st[0:HD, :])
                nc.sync.dma_start(out=qaug[0:HD, 2 * mo + 1, n2 * 512:(n2 + 1) * 512],
                                  in_=st[HD:P, :])

        if DEBUG:
            nc.sync.dma_start(
                out=d_dbg_hat.rearrange("(k p) t -> p k t", p=P), in_=hat)
            nc.sync.dma_start(out=d_dbg_k[:, :, :], in_=kaug)
            nc.sync.dma_start(out=d_dbg_q[:, :, :], in_=qaug)
            nc.sync.dma_start(out=d_dbg_v[:, :, :, :], in_=vsb)

        # =================== Phase 2.5: DSA LN + depthwise (pre-gelu) ========
        z_pool = tc.alloc_tile_pool(name="zp", bufs=1, side="left")
        z_s = z_pool.tile([P, CK, TL + 2], BF16, tag="z")

        ps_mD = psM.tile([P, 512], F32, tag="m1")
        ps_eD = psE.tile([P, 512], F32, tag="e1")
        widths = ((0, 512), (512, 512), (1024, 2))
        for kc in range(CK):
            for ci, (w0, wid) in enumerate(widths):
                xsl = xw[:, kc, w0:w0 + wid]
                sq_t = sq_pool.tile([P, 512], BF16, tag="sq")
                nc.scalar.activation(out=sq_t[:, 0:wid], in_=xsl, func=AF.Square)
                nc.tensor.matmul(ps_mD[:, 0:wid], statOH[:, ci, :], xsl,
                                 start=(kc == 0 and ci == 0),
                                 stop=(kc == CK - 1 and ci == 2),
                                 skip_group_check=True)
                nc.tensor.matmul(ps_eD[:, 0:wid], statOH[:, ci, :],
                                 sq_t[:, 0:wid],
                                 start=(kc == 0 and ci == 0),
                                 stop=(kc == CK - 1 and ci == 2),
                                 skip_group_check=True)
        rD_s, mrD_s = ln_rows(ps_mD, ps_eD)
        for ci, (w0, wid) in enumerate(widths):
            bc = bcast_rows(psB, rD_s, mrD_s, ci, wid)
            for kc in range(CK):
                t_s = tmp_pool.tile([P, 512], BF16, tag="t")
                eng.tensor_tensor(out=t_s[:, 0:wid], in0=xw[:, kc, w0:w0 + wid],
                                  in1=bc[:, 0:wid], op=OP.mult)
                eng.tensor_tensor(out=t_s[:, 0:wid], in0=t_s[:, 0:wid],
                                  in1=bc[:, 512:512 + wid], op=OP.subtract)
                eng.tensor_scalar(out=z_s[:, kc, w0:w0 + wid], in0=t_s[:, 0:wid],
                                  scalar1=dsag_s[:, kc:kc + 1],
                                  scalar2=dsab_s[:, kc:kc + 1],
                                  op0=OP.mult, op1=OP.add)
        # boundary mask: only the two halo columns can be masked out
        for kc in range(CK):
            eng.tensor_tensor(out=z_s[:, kc, 0:1], in0=z_s[:, kc, 0:1],
                              in1=mask_s[:, 0:1], op=OP.mult)
            eng.tensor_tensor(out=z_s[:, kc, TL + 1:TL + 2],
                              in0=z_s[:, kc, TL + 1:TL + 2],
                              in1=mask_s[:, 1:2], op=OP.mult)
        # depthwise 3-tap conv (bias deferred into the gelu activation)
        for kc in range(CK):
            eng.tensor_scalar(out=z1_s[:, kc, :], in0=z_s[:, kc, 0:TL],
                              scalar1=dw3_s[:, kc, 0:1], scalar2=None, op0=OP.mult)
            eng.scalar_tensor_tensor(out=z1_s[:, kc, :], in0=z_s[:, kc, 1:1 + TL],
                                     scalar=dw3_s[:, kc, 1:2],
                                     in1=z1_s[:, kc, :],
                                     op0=OP.mult, op1=OP.add)
            eng.scalar_tensor_tensor(out=z1_s[:, kc, :], in0=z_s[:, kc, 2:2 + TL],
                                     scalar=dw3_s[:, kc, 2:3],
                                     in1=z1_s[:, kc, :],
                                     op0=OP.mult, op1=OP.add)

        for pool in (z_pool, st_pool, p2, g_pool, p1, hat_pool,
                     psB, psE, psM, psA):
            pool.release()

        # late weights: load under the attention window
        late = tc.alloc_tile_pool(name="late", bufs=1, side="right")
        wo_s = late.tile([HD, H, C], BF16, tag="wo")
        nc.sync.dma_start(out=wo_s, in_=d_wo[:, :, :])
        w1_s = late.tile([P, CK, FF], BF16, tag="w1")
        nc.sync.dma_start(out=w1_s, in_=d_w1.rearrange("(k p) o -> p k o", p=P))
        w2_s = late.tile([P, FFK, C], BF16, tag="w2")
        nc.sync.dma_start(out=w2_s, in_=d_w2.rearrange("(k p) o -> p k o", p=P))
        pw_s = late.tile([P, CK, C], BF16, tag="pw")
        nc.sync.dma_start(out=pw_s, in_=d_pw.rearrange("(k p) o -> p k o", p=P))

        # =================== Phase 3: attention ===================
        attn_state = tc.alloc_tile_pool(name="attnst", bufs=1, side="right")
        attnh = attn_state.tile([HD, H, TL], BF16, tag="attnh")
        p_pool = tc.alloc_tile_pool(name="pp", bufs=4, side="left")
        vrow_t = tc.alloc_tile_pool(name="vrowt", bufs=1, side="left")
        vrow = tc.alloc_tile_pool(name="vrow", bufs=2, side="left")
        psS = tc.alloc_tile_pool(name="psS", bufs=2, space="PSUM")
        psAV = tc.alloc_tile_pool(name="psAV", bufs=2, space="PSUM")

        # heads processed in interleaved pairs. The softmax-denominator
        # finalize (psum->sbuf copies, fast reciprocal, broadcast, divide) is
        # software-pipelined: head h0's chain is emitted under h1's tail and
        # h1's broadcast+divide is deferred into the next pair, so the
        # in-order PE queue never waits on the DVE chain.
        # (reciprocal_approx_fast mis-reads upper PSUM banks -> stage the
        # denominator row through SBUF first.)
        def fin_chain(hi, ps_av_hi):
            drow = vrow_t.tile([1, 1024], F32, tag="drow", name=f"drow{hi}")
            eng.tensor_copy(out=drow, in_=ps_av_hi[HD:HD + 1, :])
            dr = vrow_t.tile([1, 1024], F32, tag="dr", name=f"dr{hi}")
            eng.reciprocal_approx_fast(out=dr, in_=drow)
            drr = vrow.tile([1, 1024], F32R, tag="drr", name=f"drr{hi}")
            eng.tensor_copy(out=drr, in_=dr)
            av_sb = vrow.tile([HD, 1024], BF16, tag="avs", name=f"avs{hi}")
            eng.tensor_copy(out=av_sb, in_=ps_av_hi[0:HD, :])
            return drr, av_sb

        def fin_apply(h, drr, av_sb):
            ps_b = psS.tile([P, 1024], F32, tag="sc")
            for n2 in range(NL):
                nc.tensor.matmul(ps_b[0:HD, n2 * 512:(n2 + 1) * 512],
                                 ones8_r[0:1, 0:HD],
                                 drr[:, n2 * 512:(n2 + 1) * 512],
                                 start=True, stop=True)
            eng.tensor_tensor(out=attnh[:, h, :], in0=av_sb,
                              in1=ps_b[0:HD, :], op=OP.mult)

        carry = []  # previous pair's finalizes, applied early next pair
        for hp in range(H // 2):
            h0 = 2 * hp
            ps_av = [psAV.tile([HD + 1, 1024], F32, tag="av", name=f"av{i}")
                     for i in range(2)]
            fin0 = None
            for tk in range(TK):
                for hi in range(2):
                    h = h0 + hi
                    ps_s = psS.tile([P, 1024], F32, tag="sc")
                    for n2 in range(NL):
                        nc.tensor.matmul(ps_s[:, n2 * 512:(n2 + 1) * 512],
                                         kaug[:, h, tk * P:(tk + 1) * P],
                                         qaug[:, h, n2 * 512:(n2 + 1) * 512],
                                         start=True, stop=True)
                    p_t = p_pool.tile([P, 1024], BF16, tag="p")
                    nc.scalar.activation(out=p_t, in_=ps_s, func=AF.Exp)
                    for n2 in range(NL):
                        nc.tensor.matmul(ps_av[hi][:, n2 * 512:(n2 + 1) * 512],
                                         vsb[:, tk, h, :],
                                         p_t[:, n2 * 512:(n2 + 1) * 512],
                                         start=(tk == 0), stop=(tk == TK - 1))
                    if tk == TK - 1 and hi == 0:
                        fin0 = fin_chain(0, ps_av[0])
                if tk in (2, 4) and carry:
                    fin_apply(*carry.pop(0))
            fin1 = fin_chain(1, ps_av[1])
            carry = [(h0,) + fin0, (h0 + 1,) + fin1]
        for c in carry:
            fin_apply(*c)
        if DEBUG:
            nc.sync.dma_start(out=d_dbg_at[:, :, :], in_=attnh)
            nc.sync.dma_start(
                out=d_dbg_z1.rearrange("(k p) t -> p k t", p=P), in_=z1_s)
        for pool in (vrow, vrow_t, p_pool, kv_state, psAV, psS):
            pool.release()

        # =================== Phase 4: out-proj + residual + LN2 ==============
        ftc2_pool = tc.alloc_tile_pool(name="ftc2p", bufs=1, side="right")
        ftc2 = ftc2_pool.tile([P, CK, TL], BF16, tag="ftc2")
        psC = tc.alloc_tile_pool(name="psC", bufs=2, space="PSUM")
        psB2 = tc.alloc_tile_pool(name="psB2", bufs=1, space="PSUM")
        psM2 = tc.alloc_tile_pool(name="psM2", bufs=1, space="PSUM")
        psE2 = tc.alloc_tile_pool(name="psE2", bufs=1, space="PSUM")

        ps_m2 = psM2.tile([P, 512], F32, tag="m2")
        ps_e2 = psE2.tile([P, 512], F32, tag="e2")

        def emit_stats2(idx, sq_t):
            n2, mo = idx
            cc = slice(n2 * 512, (n2 + 1) * 512)
            nc.tensor.matmul(ps_m2, statOH[:, n2, :], ftc2[:, mo, cc],
                             start=(n2 == 0 and mo == 0),
                             stop=(n2 == NL - 1 and mo == CK - 1))
            nc.tensor.matmul(ps_e2, statOH[:, n2, :], sq_t,
                             start=(n2 == 0 and mo == 0),
                             stop=(n2 == NL - 1 and mo == CK - 1))

        pend2 = []
        for n2 in range(NL):
            cc = slice(n2 * 512, (n2 + 1) * 512)
            for mo in range(CK):
                ps_o = psC.tile([P, 512], F32, tag="mm")
                for h in range(H):
                    nc.tensor.matmul(ps_o, wo_s[:, h, mo * P:(mo + 1) * P],
                                     attnh[:, h, cc],
                                     start=(h == 0), stop=(h == H - 1))
                if len(pend2) >= 2:
                    emit_stats2(*pend2.pop(0))
                eng.scalar_tensor_tensor(
                    out=ftc2[:, mo, cc], in0=ps_o, scalar=ob_s[:, mo:mo + 1],
                    in1=xw[:, mo, 1 + n2 * 512:1 + (n2 + 1) * 512],
                    op0=OP.add, op1=OP.add)
                sq_t = sq_pool.tile([P, 512], BF16, tag="sq")
                nc.scalar.activation(out=sq_t, in_=ftc2[:, mo, cc], func=AF.Square)
                pend2.append(((n2, mo), sq_t))
        for args in pend2:
            emit_stats2(*args)
        r2_s, mr2_s = ln_rows(ps_m2, ps_e2)
        for pool in (psE2, psM2):
            pool.release()

        # ========= Phase 5 fused per local-half: normalize -> MLP ==========
        z1g_pool = tc.alloc_tile_pool(name="z1gp", bufs=1, side="right")
        z1g = z1g_pool.tile([P, CK, TL], BF16, tag="z1g")
        for kc in range(CK):
            nc.scalar.activation(out=z1g[:, kc, :], in_=z1_s[:, kc, :],
                                 func=AF.Gelu, bias=dsadb_s[:, kc:kc + 1])

        hh_pool = tc.alloc_tile_pool(name="hh", bufs=3, side="right")
        fin_pool = tc.alloc_tile_pool(name="fin", bufs=3, side="right")
        psO = tc.alloc_tile_pool(name="psO", bufs=1, space="PSUM")
        for n2 in range(NL):
            cc = slice(n2 * 512, (n2 + 1) * 512)
            bc = bcast_rows(psB2, r2_s, mr2_s, n2, 512)
            for kc in range(CK):
                t_s = tmp_pool.tile([P, 512], BF16, tag="t")
                eng.tensor_tensor(out=t_s, in0=ftc2[:, kc, cc],
                                  in1=bc[:, 0:512], op=OP.mult)
                eng.tensor_tensor(out=ftc2[:, kc, cc], in0=t_s,
                                  in1=bc[:, 512:1024], op=OP.subtract)
            ps_out = [psO.tile([P, 512], F32, tag=f"out{mo}", name=f"psout{mo}")
                      for mo in range(CK)]
            for ff in range(FFK):
                ps_h = psC.tile([P, 512], F32, tag="mm")
                for kc in range(CK):
                    nc.tensor.matmul(ps_h, w1_s[:, kc, ff * P:(ff + 1) * P],
                                     ftc2[:, kc, cc],
                                     start=(kc == 0), stop=(kc == CK - 1))
                hh_t = hh_pool.tile([P, 512], BF16, tag="hh")
                nc.scalar.activation(out=hh_t, in_=ps_h, func=AF.Gelu,
                                     bias=b1_s[:, ff:ff + 1])
                for mo in range(CK):
                    nc.tensor.matmul(ps_out[mo], w2_s[:, ff, mo * P:(mo + 1) * P],
                                     hh_t, start=(ff == 0), stop=False)
            # fold the DSA pointwise conv into the same accumulators
            for mo in range(CK):
                for kc in range(CK):
                    nc.tensor.matmul(ps_out[mo], pw_s[:, kc, mo * P:(mo + 1) * P],
                                     z1g[:, kc, cc],
                                     start=False, stop=(kc == CK - 1))
            for mo in range(CK):
                fin_t = fin_pool.tile([P, 512], F32, tag="fin")
                eng.tensor_scalar(out=fin_t, in0=ps_out[mo],
                                  scalar1=bfin_s[:, mo:mo + 1], scalar2=None,
                                  op0=OP.add)
                nc.sync.dma_start(out=d_out[mo * P:(mo + 1) * P, cc], in_=fin_t)

        for pool in (fin_pool, hh_pool, z1g_pool, ftc2_pool, attn_state,
                     late, z1_pool, dsa_pool, sq_pool, bc_pool, tmp_pool,
                     row_pool, consts, psO, psB2, psC):
            pool.release()

    nc.compile()
    return nc


def _in_maps(inputs):
    f = lambda v: np.ascontiguousarray(np.asarray(v), dtype=np.float32)
    bf = lambda v: np.ascontiguousarray(np.asarray(v, dtype=np.float32).astype(BF))
    x = f(inputs["x"])            # [B, T, C]
    A = f(inputs["A"])            # [B, T]
    alpha = float(np.asarray(inputs["alpha_bias"]).reshape(-1)[0])
    dst_a = float(np.asarray(inputs["dst_alpha"]))
    dst_b = float(np.asarray(inputs["dst_beta"]))
    conv1_w, conv1_b = f(inputs["conv1_w"]), f(inputs["conv1_b"])
    ln1_g, ln1_b = f(inputs["ln1_g"]), f(inputs["ln1_b"])
    in_w, in_b = f(inputs["in_proj_w"]), f(inputs["in_proj_b"])
    out_w, out_b = f(inputs["out_w"]), f(inputs["out_b"])
    ln2_g, ln2_b = f(inputs["ln2_g"]), f(inputs["ln2_b"])
    w1, b1 = f(inputs["mlp_w1"]), f(inputs["mlp_b1"])
    w2, b2 = f(inputs["mlp_w2"]), f(inputs["mlp_b2"])
    dsa_g, dsa_b = f(inputs["dsa_ln_g"]), f(inputs["dsa_ln_b"])
    dsa_dw, dsa_db = f(inputs["dsa_dw"]), f(inputs["dsa_db"])
    dsa_pw, dsa_pb = f(inputs["dsa_pw"]), f(inputs["dsa_pb"])

    weff = in_w * ln1_g[None, :]
    beff = in_w @ ln1_b + in_b
    assert np.abs(beff[:2 * C]).max() < 1e-6, "nonzero q/k bias not supported"
    weff[:C] /= np.sqrt(HD).astype(np.float32)
    beff[:C] /= np.sqrt(HD).astype(np.float32)

    # one-hot (1/C) stationary columns: chunk j's stats land on psum
    # partition 32*j (matmul base partitions must be 32-aligned)
    statOH = np.zeros((P, 4, P), np.float32)
    for j in range(4):
        statOH[:, j, 32 * j] = 1.0 / C
    shared = {
        "convw": bf(np.transpose(conv1_w, (2, 1, 0))),
        "convb": conv1_b,
        "wqkv": bf(weff.T),
        "wo": bf(out_w.T.reshape(HD * H, C).reshape(H, HD, C).transpose(1, 0, 2)),
        # softmax weights sum to 1, so the V bias rides through attention
        # additively and folds exactly into the out-proj bias
        "ob": out_w @ beff[2 * C:3 * C] + out_b,
        "w1": bf((w1 * ln2_g[None, :]).T),
        "b1": w1 @ ln2_b + b1,
        "w2": bf((dst_a * w2).T),
        "bfin": dst_a * b2 + dst_b * dsa_pb,
        "pw": bf((dst_b * dsa_pw[:, :, 0]).T),
        "dsag": dsa_g, "dsab": dsa_b,
        "dw3": dsa_dw[:, 0, :], "dsadb": dsa_db,
        "statOH": statOH.astype(BF),
        "ones8": np.ones((P, P), np.float32),
        "eps8": np.full((P, 1), 1e-5, np.float32),
    }
    maps = []
    for core in range(8):
        b, half = core // 2, core % 2
        j0 = half * TL
        xT = np.zeros((C, T + 2), np.float32)
        xT[:, 1:T + 1] = x[b].T
        xd = np.zeros((C, TL + 2), np.float32)
        lo, hi = j0 - 1, j0 + TL + 1
        slo, shi = max(lo, 0), min(hi, T)
        xd[:, slo - lo:slo - lo + (shi - slo)] = x[b].T[:, slo:shi]
        mask = np.ones((1, 2), np.float32)
        if lo < 0:
            mask[0, 0] = 0.0
        if hi > T:
            mask[0, 1] = 0.0
        m = dict(shared)
        m["xT"] = xT.astype(BF)
        m["xd"] = xd.astype(BF)
        m["maskbc"] = np.ascontiguousarray(np.broadcast_to(mask, (P, 2))).astype(BF)
        m["Arow"] = A[b:b + 1, :].astype(BF)
        m["qArow"] = (alpha * A[b:b + 1, j0:j0 + TL]).astype(BF)
        m["qoff"] = np.array([[j0]], np.uint32)
        maps.append(m)
    return maps


def _get_program():
    global _CACHED
    if _CACHED is None:
        _CACHED = _build()
    return _CACHED


def kernel(**inputs):
    nc = _get_program()
    maps = _in_maps(inputs)
    res = run_bass_kernel_spmd(nc, maps, list(range(8)))
    out = np.empty((B, T, C), np.float32)
    for core in range(8):
        b, half = core // 2, core % 2
        out[b, half * TL:(half + 1) * TL, :] = res.results[core]["outT"].T
    return out
